# revision 1
# baseline (speedup 1.0000x reference)
"""Trainium2 Bass kernel v3 for CHAI sparse attention.

Per-core layout (8-core SPMD, one NEFF):
  - dual-slot balanced heads: each core owns a 3-head chunk of cluster
    cA plus a 1-head chunk of cluster cB (every core exactly 4 heads,
    V-projection width 512). This keeps the attention AllGather at
    0.5 MB/rank per half, below the slow-algorithm cliff measured at
    0.75 MB/rank (80 us vs 20 us per AllGather).
  - all projections local (qA,kA,qB,kB); the A-pair rides the x DMA
    wave (weights DMA'd first), exp(scores_A) on ACT overlaps the
    B-pair projections on PE.
  - per-sb interleave: V(sb) + scores_B one block ahead + AV(sb) +
    transpose; attention output AllGathered per 256-row sequence
    quarter as soon as its rows are done (2 MB gathered, measured well
    below the slow-collective cliff); wo matmul per quarter behind
    each AG.
  - all weight/x DMAs host-prepacked into device layout [P, NB*X] so
    every big transfer is one contiguous per-partition segment; the AG
    wire format is one partition-block per rank so gathered reads are
    2 KB segments.
"""

import math
import numpy as np

import concourse.bass as bass
import concourse.bacc as bacc
import concourse.tile as tile
import concourse.mybir as mybir
from concourse.masks import make_identity

P = 128
S = 1024
D = 4096
HD = 128
H = 32
C = 8
NB = D // P        # 32 blocks over the D contraction
SB = S // P        # 8 blocks over sequence
OC = 512           # output columns per core
NCORES = 8
NEG = -1e9
QW = 256           # AG chunk width over sequence (4 quarters)
NQ = S // QW
SBQ = QW // P      # sb blocks per AG chunk

WA = 384           # A-slot attn width (3 heads)
WB = 128           # B-slot attn width
OA = WA            # ones column for A at 384
OB = WA + 1 + WB   # ones column for B at 513
VW = WA + WB + 2   # 514

F32 = mybir.dt.float32
BF16 = mybir.dt.bfloat16


def _decompose_31(nper):
    """Split clusters into 8 three-head chunks + 8 one-head chunks."""
    a = [n // 3 for n in nper]
    total = sum(a)
    if total < 8:
        return None
    cs_ = list(range(len(nper)))
    i = 0
    while total > 8:
        c = cs_[i % len(cs_)]
        if a[c] > 0:
            a[c] -= 1
            total -= 1
        i += 1
    b = [nper[c] - 3 * a[c] for c in range(len(nper))]
    if sum(b) != 8 or any(x < 0 for x in b):
        return None
    threes, ones = [], []
    for c in range(len(nper)):
        threes += [c] * a[c]
        ones += [c] * b[c]
    if len(threes) != 8 or len(ones) != 8:
        return None
    return threes, ones


def prep(inputs, compute_in_bf16_io=True):
    x = np.asarray(inputs["x"], np.float32)
    wq = np.asarray(inputs["wq"], np.float32)
    wk = np.asarray(inputs["wk"], np.float32)
    wv = np.asarray(inputs["wv"], np.float32)
    wo = np.asarray(inputs["wo"], np.float32)
    cos = np.asarray(inputs["freqs_cos"], np.float32)
    sin = np.asarray(inputs["freqs_sin"], np.float32)
    mask = np.asarray(inputs["mask"], np.float32)
    lab = np.asarray(inputs["cluster_assignment"]).astype(np.int64)
    rep = np.asarray(inputs["rep_heads"]).astype(np.int64)

    members = [[h for h in range(H) if lab[h] == c] for c in range(C)]
    nper = [len(m) for m in members]
    dec = _decompose_31(nper)
    assert dec is not None, "3+1 decomposition failed for this input"
    threes, ones = dec

    cursor = {c: 0 for c in range(C)}

    def take(c, k):
        i0 = cursor[c]
        cursor[c] += k
        return members[c][i0:i0 + k]

    slotsA = [(c, take(c, 3)) for c in threes]
    slotsB = [(c, take(c, 1)) for c in ones]
    assert all(cursor[c] == nper[c] for c in range(C))

    causal_mask = np.where(np.triu(np.ones((S, S), bool), k=1),
                           np.float32(NEG), np.float32(0.0))
    causal = np.array_equal(mask, causal_mask)

    import ml_dtypes
    bf = ml_dtypes.bfloat16

    def cvt(a):
        return np.ascontiguousarray(a).astype(bf)

    # device-layout packing: [P, NB*X] with row d = b*P + p, so every
    # big DMA is one contiguous per-partition segment
    def devpack(a):
        # a: [D, X] -> [P, (D//P)*X]
        X = a.shape[1]
        return np.ascontiguousarray(
            a.reshape(NB, P, X).transpose(1, 0, 2).reshape(P, NB * X))

    xT = cvt(devpack(x[0].T))
    cs = np.empty((HD, S), np.float32)
    sn = np.empty((HD, S), np.float32)
    cs[0::2] = cos.T
    cs[1::2] = cos.T
    sn[0::2] = -sin.T
    sn[1::2] = sin.T
    psw = np.zeros((HD, HD), np.float32)
    for i in range(HD):
        psw[i, i ^ 1] = 1.0
    kk = np.arange(P)[:, None]
    qq = np.arange(P)[None, :]
    tri = np.where(kk > qq, np.float32(NEG), np.float32(0.0))

    # global gathered block order: rank i contributes [A0,A1,A2,B0]
    scale = 1.0 / math.sqrt(HD)
    all_heads = []
    for i in range(NCORES):
        all_heads += slotsA[i][1] + slotsB[i][1]
    assert sorted(all_heads) == list(range(H))

    in_maps = []
    for i in range(NCORES):
        cA, headsA = slotsA[i]
        cB, headsB = slotsB[i]
        wqkA = np.empty((D, 2 * HD), np.float32)
        wqkA[:, 0:HD] = wq[rep[cA] * HD:(rep[cA] + 1) * HD, :].T * scale
        wqkA[:, HD:2 * HD] = wk[rep[cA] * HD:(rep[cA] + 1) * HD, :].T
        wqkB = np.empty((D, 2 * HD), np.float32)
        wqkB[:, 0:HD] = wq[rep[cB] * HD:(rep[cB] + 1) * HD, :].T * scale
        wqkB[:, HD:2 * HD] = wk[rep[cB] * HD:(rep[cB] + 1) * HD, :].T
        wvT = np.empty((D, 4 * HD), np.float32)
        for j, h in enumerate(headsA + headsB):
            wvT[:, j * P:(j + 1) * P] = wv[h * HD:(h + 1) * HD, :].T
        woT = np.empty((H * P, OC), np.float32)
        for r, h in enumerate(all_heads):
            woT[r * P:(r + 1) * P, :] = wo[i * OC:(i + 1) * OC,
                                           h * HD:(h + 1) * HD].T
        woTd = np.ascontiguousarray(
            woT.reshape(H, P, OC).transpose(1, 0, 2).reshape(P, H * OC))
        m = {
            "xT": xT, "wqkAT": cvt(devpack(wqkA)),
            "wqkBT": cvt(devpack(wqkB)), "wvT": cvt(devpack(wvT)),
            "woT": cvt(woTd),
            "cs": cs, "sn": sn, "psw": psw, "tri": tri,
        }
        if not causal:
            m["maskT"] = np.ascontiguousarray(mask.T)
        in_maps.append(m)

    meta = {"causal": causal, "bf16_io": True}
    return in_maps, meta


def assemble(results, meta):
    out = np.empty((1, S, D), np.float32)
    for c in range(NCORES):
        out[0, :, c * OC:(c + 1) * OC] = results[c]["out"]
    return out


def emit_body(nc, tc, tensors, meta, body_idx=0, chain=False):
    causal = meta["causal"]
    xT, wqkAT, wqkBT, wvT, woT = (
        tensors[k] for k in ("xT", "wqkAT", "wqkBT", "wvT", "woT"))
    cs, sn, psw, tri = (tensors[k] for k in ("cs", "sn", "psw", "tri"))
    out = tensors["out"]
    maskT = tensors.get("maskT")

    XG = 8
    GB = NB // XG

    with (
        tc.tile_pool(name="const", bufs=1) as constp,
        tc.tile_pool(name="persist", bufs=1) as persist,
        tc.tile_pool(name="ph1w", bufs=1) as ph1w,
        tc.tile_pool(name="dram", bufs=1, space="DRAM") as dram,
    ):
        ident = constp.tile([P, P], BF16)
        make_identity(nc, ident[:])
        trib = constp.tile([P, P], F32)
        nc.sync.dma_start(trib[:], tri[:])

        qk_dT = [persist.tile([P, S], BF16, name=f"qk{i}_{body_idx}")
                 for i in range(4)]    # qA, kA, qB, kB
        Vs = [persist.tile([P, VW], BF16, name=f"V{kb}_{body_idx}")
              for kb in range(SB)]
        # pT tiles shrunk to the causal width (col c maps to q = kb*P + c)
        pTw = [(S - kb * P if causal else S) for kb in range(SB)]
        pTa = [persist.tile([P, pTw[kb]], BF16, name=f"pTa{kb}_{body_idx}")
               for kb in range(SB)]
        pTb = [persist.tile([P, pTw[kb]], BF16, name=f"pTb{kb}_{body_idx}")
               for kb in range(SB)]

        junk_b = None
        trunc_row = (body_idx % 2) * P
        prev_row = ((body_idx - 1) % 2) * P
        jrow = prev_row if meta.get("trunc") else 0
        if chain and body_idx > 0:
            junk = constp.tile([P, 16], F32, name="junk")
            nc.sync.dma_start(junk[:], out[jrow:jrow + P, 0:16])
            junk_b = constp.tile([P, 16], BF16, name="junkb")
            nc.vector.tensor_scalar_mul(junk_b[:], junk[:], 0.0)

        def chain_dep(ap2d):
            if junk_b is not None:
                nc.vector.tensor_copy(ap2d, junk_b[:])

        def tiny_ag(pool, src_bf16):
            # serialize the body through a small collective: bounce ->
            # AllGather -> read back -> out write (anchors against DCE
            # and cross-body overlap)
            bt = dram.tile([P, 512], BF16, name=f"tb{body_idx}")
            gt = dram.tile([NCORES * P, 512], BF16, addr_space="Shared",
                           name=f"tg{body_idx}")
            nc.sync.dma_start(bt[:], src_bf16)
            nc.gpsimd.collective_compute(
                "AllGather", mybir.AluOpType.bypass,
                replica_groups=[list(range(NCORES))],
                ins=[bt[:].opt()], outs=[gt[:].opt()])
            rb = pool.tile([P, 512], BF16, tag="tagrb")
            nc.sync.dma_start(rb[:], gt[(NCORES - 1) * P:NCORES * P, :])
            tf = pool.tile([P, 512], F32, tag="truncf")
            nc.vector.tensor_copy(tf[:], rb[:])
            nc.sync.dma_start(out[trunc_row:trunc_row + P, :], tf[:])

        if meta.get("trunc") == "agonly":
            with tc.tile_pool(name="tagp", bufs=1) as tagp:
                t0_ = tagp.tile([P, 512], BF16, tag="t0")
                nc.vector.memset(t0_[:], 0.25)
                chain_dep(t0_[:, :16])
                tiny_ag(tagp, t0_[:])
            return

        # ================= projections (scoped pools) ===================
        with (
            tc.tile_pool(name="wqkp", bufs=1) as wqkp,
            tc.tile_pool(name="ropep", bufs=2) as ropep,
            tc.tile_pool(name="psproj", bufs=1, space="PSUM") as psprojp,
            tc.tile_pool(name="psrope", bufs=2, space="PSUM") as psropep,
        ):
            wqkAb = wqkp.tile([P, NB, 2 * HD], BF16)
            wqkBb = wqkp.tile([P, NB, 2 * HD], BF16)
            chain_dep(wqkAb[:, 0, :16])
            chain_dep(wqkBb[:, 0, :16])
            wvb = ph1w.tile([P, NB, 4 * HD], BF16)
            chain_dep(wvb[:, 0, :16])
            xbs = []
            for g in range(XG):
                # interleave A-pair weight chunks with x groups so the
                # first projection matmul can start after ~1.3 MB of DMA
                nc.sync.dma_start(
                    wqkAb[:, g * GB:(g + 1) * GB, :],
                    wqkAT[:, g * GB * 2 * HD:(g + 1) * GB * 2 * HD])
                xg = ph1w.tile([P, GB, S], BF16, name=f"xb{g}")
                chain_dep(xg[:, 0, :16])
                nc.sync.dma_start(
                    xg[:], xT[:, g * GB * S:(g + 1) * GB * S])
                xbs.append(xg)
                if g == 0:
                    csb = ropep.tile([P, S], F32, tag="csb")
                    nc.sync.dma_start(csb[:], cs[:])
                    snb = ropep.tile([P, S], F32, tag="snb")
                    nc.sync.dma_start(snb[:], sn[:])
                    pswf = ropep.tile([P, P], F32, tag="pswf")
                    nc.sync.dma_start(pswf[:], psw[:])
                    pswb = ropep.tile([P, P], BF16, tag="pswb")
                    nc.vector.tensor_copy(pswb[:], pswf[:])
                if g == 3:
                    nc.sync.dma_start(wqkBb[:], wqkBT[:, :])
                nc.sync.dma_start(
                    wvb[:, g * GB:(g + 1) * GB, :],
                    wvT[:, g * GB * 4 * HD:(g + 1) * GB * 4 * HD])

            def xb(db):
                return xbs[db // GB][:, db % GB, :]

            def proj_pair(base):
                # base=0 -> qA,kA ; base=2 -> qB,kB. db-outer so the
                # matmuls start as soon as the first x group lands.
                wb = wqkAb if base == 0 else wqkBb
                pss = [psprojp.tile([P, 512], F32, tag=f"pp{j}",
                                    name=f"pp{base}_{j}_{body_idx}")
                       for j in range(4)]  # q ch0, q ch1, k ch0, k ch1
                for db in range(NB):
                    for pj in range(2):       # q, k
                        for ch in range(2):
                            nc.tensor.matmul(
                                pss[pj * 2 + ch][:],
                                wb[:, db, pj * HD:(pj + 1) * HD],
                                xb(db)[:, ch * 512:(ch + 1) * 512],
                                start=(db == 0), stop=(db == NB - 1))
                return pss

            def rope(pss, base):
                for pj in range(2):
                    dst = qk_dT[base + pj]
                    raw = ropep.tile([P, S], BF16, tag="qraw")
                    for ch in range(2):
                        nc.vector.tensor_copy(raw[:, ch * 512:(ch + 1) * 512],
                                              pss[pj * 2 + ch][:])
                    for ch in range(2):
                        csl = slice(ch * 512, (ch + 1) * 512)
                        ps2 = psropep.tile([P, 512], F32, tag="pswp")
                        nc.tensor.matmul(ps2[:], pswb[:], raw[:, csl],
                                         start=True, stop=True)
                        t1 = ropep.tile([P, 512], F32, tag="rope1")
                        nc.vector.tensor_tensor(t1[:], ps2[:], snb[:, csl],
                                                op=mybir.AluOpType.mult)
                        t2 = ropep.tile([P, 512], F32, tag="rope2")
                        nc.vector.tensor_tensor(t2[:], raw[:, csl],
                                                csb[:, csl],
                                                op=mybir.AluOpType.mult)
                        nc.vector.tensor_tensor(dst[:, csl], t1[:], t2[:],
                                                op=mybir.AluOpType.add)

            pssA = proj_pair(0)
            rope(pssA, 0)
            pssB = proj_pair(2)
            rope(pssB, 2)
            if meta.get("trunc") == "proj":
                tiny_ag(ropep, qk_dT[3][:, 0:512])
                return

        # ============== scores_A + interleaved V/scores_B/AV ============
        agouts = [None] * NQ
        with (
            tc.tile_pool(name="ph3", bufs=2) as ph3,
            tc.tile_pool(name="aTp", bufs=2) as aTp,
            tc.tile_pool(name="psv", bufs=2, space="PSUM") as psvp,
            tc.tile_pool(name="pssc", bufs=2, space="PSUM") as psscp,
            tc.tile_pool(name="pav", bufs=1, space="PSUM") as pavp,
            tc.tile_pool(name="ptr", bufs=2, space="PSUM") as ptrp,
        ):
            def scores(qi, ki_, pTs, kb):
                q0 = kb * P if causal else 0
                if maskT is not None:
                    mrow = ph3.tile([P, S], F32, tag="mrow")
                    nc.sync.dma_start(mrow[:], maskT[kb * P:(kb + 1) * P, :])
                c0 = q0
                while c0 < S:
                    c1 = min(c0 + 512, S)
                    ps = psscp.tile([P, 512], F32, tag="psc")
                    nc.tensor.matmul(ps[:, :c1 - c0],
                                     qk_dT[ki_][:, kb * P:(kb + 1) * P],
                                     qk_dT[qi][:, c0:c1],
                                     start=True, stop=True)
                    if maskT is not None:
                        nc.vector.tensor_tensor(
                            ps[:, :c1 - c0], ps[:, :c1 - c0],
                            mrow[:, c0:c1], op=mybir.AluOpType.add)
                    elif causal and c0 == q0:
                        nc.vector.tensor_tensor(
                            ps[:, :P], ps[:, :P], trib[:],
                            op=mybir.AluOpType.add)
                    nc.scalar.activation(
                        pTs[kb][:, c0 - (kb * P if causal else 0):
                                c1 - (kb * P if causal else 0)],
                        ps[:, :c1 - c0],
                        mybir.ActivationFunctionType.Exp)
                    c0 = c1

            for kb in range(SB):
                scores(0, 1, pTa, kb)
            scores(2, 3, pTb, 0)

            aTt = None
            for sb in range(SB):
                # V projection for this sequence block
                psv = psvp.tile([P, 512], F32, tag="pv")
                for db in range(NB):
                    nc.tensor.matmul(
                        psv[:], xb(db)[:, sb * P:(sb + 1) * P],
                        wvb[:, db, :],
                        start=(db == 0), stop=(db == NB - 1))
                nc.vector.tensor_copy(Vs[sb][:, 0:WA], psv[:, 0:WA])
                nc.vector.tensor_copy(Vs[sb][:, OA + 1:OA + 1 + WB],
                                      psv[:, WA:WA + WB])
                nc.vector.memset(Vs[sb][:, OA:OA + 1], 1.0)
                nc.vector.memset(Vs[sb][:, OB:OB + 1], 1.0)

                # B scores one block ahead (hide the exp latency)
                if sb + 1 < SB:
                    scores(2, 3, pTb, sb + 1)

                # AV for q block sb (causal: ki <= sb)
                if sb % SBQ == 0:
                    aTt = aTp.tile([P, 4, QW], BF16, tag="aT",
                                   name=f"aT{sb // SBQ}_{body_idx}")
                kmax = sb + 1 if causal else SB
                psA = pavp.tile([P, WA + 1], F32, tag="pava",
                                name=f"pava{sb}_{body_idx}")
                psB = pavp.tile([P, WB + 1], F32, tag="pavb",
                                name=f"pavb{sb}_{body_idx}")
                for ki in range(kmax):
                    o = ki * P if causal else 0
                    ltA = pTa[ki][:, sb * P - o:(sb + 1) * P - o]
                    ltB = pTb[ki][:, sb * P - o:(sb + 1) * P - o]
                    nc.tensor.matmul(psA[:], ltA, Vs[ki][:, 0:WA + 1],
                                     start=(ki == 0), stop=(ki == kmax - 1))
                    nc.tensor.matmul(psB[:], ltB, Vs[ki][:, OA + 1:OB + 1],
                                     start=(ki == 0), stop=(ki == kmax - 1))
                rLA = ph3.tile([P, 1], F32, tag="rLA")
                nc.vector.reciprocal(rLA[:], psA[:, WA:WA + 1])
                rLB = ph3.tile([P, 1], F32, tag="rLB")
                nc.vector.reciprocal(rLB[:], psB[:, WB:WB + 1])
                attn = ph3.tile([P, 512], BF16, tag="attn")
                nc.vector.tensor_scalar_mul(attn[:, 0:WA], psA[:, :WA],
                                            rLA[:])
                nc.vector.tensor_scalar_mul(attn[:, WA:512], psB[:, :WB],
                                            rLB[:])
                qtr, sl = sb // SBQ, (sb % SBQ) * P
                for blk in range(4):
                    pst = ptrp.tile([P, P], BF16, tag="ptr")
                    nc.tensor.transpose(pst[:], attn[:, blk * P:(blk + 1) * P],
                                        ident[:])
                    nc.vector.tensor_copy(aTt[:, blk, sl:sl + P], pst[:])

                if meta.get("trunc") == "phase3":
                    if sb == SB - 1:
                        tiny_ag(ph3, attn[:])
                    continue
                # kick off the AllGather as soon as a chunk is complete
                if sb % SBQ == SBQ - 1:
                    b = dram.tile([P, 4 * QW], BF16,
                                  name=f"bounce{body_idx}_{qtr}")
                    g = dram.tile([NCORES * P, 4 * QW], BF16,
                                  addr_space="Local" if meta.get("nocoll")
                                  else "Shared",
                                  name=f"agout{body_idx}_{qtr}")
                    agouts[qtr] = g
                    nc.sync.dma_start(
                        b[:].rearrange("p (b f) -> p b f", b=4), aTt[:])
                    if meta.get("nocoll"):
                        for cc_ in range(NCORES):
                            nc.sync.dma_start(
                                g[cc_ * P:(cc_ + 1) * P, :], b[:])
                    else:
                        nc.gpsimd.collective_compute(
                            "AllGather", mybir.AluOpType.bypass,
                            replica_groups=[list(range(NCORES))],
                            ins=[b[:].opt()], outs=[g[:].opt()])

        if meta.get("trunc") in ("proj", "phase3"):
            return
        if meta.get("trunc") == "nowo":
            # keep the AllGathers (serialization anchors) but skip the wo
            # matmuls: reduce a slice of each gathered buffer into out
            with tc.tile_pool(name="nwp", bufs=1) as nwp:
                acc = nwp.tile([P, 512], BF16, tag="acc")
                nc.vector.memset(acc[:], 0.0)
                for qtr in range(NQ):
                    rb = nwp.tile([P, QW], BF16, tag=f"rb{qtr}")
                    nc.sync.dma_start(
                        rb[:], agouts[qtr][(NCORES - 1) * P:NCORES * P,
                                           0:QW])
                    nc.vector.tensor_tensor(acc[:, 0:QW], acc[:, 0:QW],
                                            rb[:], op=mybir.AluOpType.add)
                tf = nwp.tile([P, 512], F32, tag="truncf")
                nc.vector.tensor_copy(tf[:], acc[:])
                nc.sync.dma_start(out[trunc_row:trunc_row + P, :], tf[:])
            return
        # ================= wo matmul per AG chunk =======================
        with (
            tc.tile_pool(name="ph5w", bufs=1) as ph5w,
            tc.tile_pool(name="ph5", bufs=1) as ph5,
            tc.tile_pool(name="po", bufs=1, space="PSUM") as pop,
        ):
            wob = ph5w.tile([P, H, OC], BF16)
            nc.sync.dma_start(wob[:], woT[:, :])

            nsb = QW // P
            NH = NCORES // 2
            for qtr in range(NQ):
                g = agouts[qtr]
                agts = []
                for hf in range(2):
                    agt = ph5.tile([P, NH, 4 * QW], BF16, tag=f"agt{hf}",
                                   name=f"agt{qtr}_{hf}_{body_idx}")
                    nc.sync.dma_start(
                        agt[:], g[hf * NH * P:(hf + 1) * NH * P, :]
                        .rearrange("(b p) f -> p b f", p=P))
                    agts.append(agt)
                pos = [pop.tile([P, OC], F32, tag=f"po{i}",
                                name=f"po{i}_{qtr}_{body_idx}")
                       for i in range(nsb)]
                r = 0
                for j in range(NCORES):
                    agt = agts[j // NH]
                    for blk in range(4):
                        for i in range(nsb):
                            nc.tensor.matmul(
                                pos[i][:],
                                agt[:, j % NH, blk * QW + i * P:
                                    blk * QW + (i + 1) * P],
                                wob[:, r, :],
                                start=(r == 0), stop=(r == H - 1))
                        r += 1
                otile = ph5.tile([P, nsb, OC], F32, tag="ot",
                                 name=f"ot{qtr}_{body_idx}")
                for i in range(nsb):
                    nc.vector.tensor_copy(otile[:, i, :], pos[i][:])
                nc.sync.dma_start(
                    out[qtr * QW:(qtr + 1) * QW, :]
                    .rearrange("(b p) o -> p b o", p=P), otile[:])


def build_kernel(meta, repeat=1, chain=True):
    nc = bacc.Bacc("TRN2", target_bir_lowering=False, debug=False,
                   num_devices=NCORES)
    tensors = {
        "xT": nc.dram_tensor("xT", [P, NB * S], BF16, kind="ExternalInput"),
        "wqkAT": nc.dram_tensor("wqkAT", [P, NB * 2 * HD], BF16,
                                kind="ExternalInput"),
        "wqkBT": nc.dram_tensor("wqkBT", [P, NB * 2 * HD], BF16,
                                kind="ExternalInput"),
        "wvT": nc.dram_tensor("wvT", [P, NB * 4 * HD], BF16,
                              kind="ExternalInput"),
        "woT": nc.dram_tensor("woT", [P, H * OC], BF16,
                              kind="ExternalInput"),
        "cs": nc.dram_tensor("cs", [HD, S], F32, kind="ExternalInput"),
        "sn": nc.dram_tensor("sn", [HD, S], F32, kind="ExternalInput"),
        "psw": nc.dram_tensor("psw", [HD, HD], F32, kind="ExternalInput"),
        "tri": nc.dram_tensor("tri", [P, P], F32, kind="ExternalInput"),
        "out": nc.dram_tensor("out", [S, OC], F32, kind="ExternalOutput"),
    }
    if not meta["causal"]:
        tensors["maskT"] = nc.dram_tensor("maskT", [S, S], F32,
                                          kind="ExternalInput")
    with tile.TileContext(nc) as tc:
        if repeat == 0:
            with tc.tile_pool(name="z", bufs=1) as zp:
                zt = zp.tile([P, SB, OC], F32)
                nc.vector.memset(zt[:], 0.0)
                nc.sync.dma_start(
                    tensors["out"][:].rearrange("(b p) o -> p b o", p=P),
                    zt[:])
        else:
            for r in range(repeat):
                emit_body(nc, tc, tensors, meta, body_idx=r, chain=chain)
    nc.compile()
    return nc


def kernel(**inputs):
    import numpy as _np
    np_inputs = {k: (_np.asarray(v) if not _np.isscalar(v) else v)
                 for k, v in inputs.items()}
    in_maps, meta = prep(np_inputs)
    nc = build_kernel(meta, repeat=1, chain=False)
    from concourse import bass_utils
    res = bass_utils.run_bass_kernel_spmd(
        nc, in_maps, core_ids=list(range(NCORES)))
    return assemble(res.results, meta)



# revision 13
# speedup vs baseline: 4.1714x; 4.1714x over previous
"""Trainium2 Bass kernel v4 for CHAI sparse attention.

Per-core layout (8-core SPMD, one NEFF):
  - dual-slot balanced heads: each core owns a 3-head chunk of cluster
    cA plus a 1-head chunk of cluster cB (every core exactly 4 heads).
  - all projections local (qA,kA,qB,kB); exp(scores_A) on ACT overlaps
    the B-pair projections on PE.
  - per-sb interleave: V(sb) + scores_B one block ahead + AV(sb) +
    transpose into a persistent aT_all tile.
  - NO AllGather of attention outputs. Instead each core computes wo
    PARTIALS over all 4096 output columns (contracting only its own 4
    heads' 512 dims — same PE cost as the old gathered scheme), then a
    chunked ReduceScatter sums partials across cores; rank c keeps rows
    c*SH..(c+1)*SH of each chunk. This removes every per-quarter
    cross-core barrier and the 8MB/core gathered-read DMA; the only
    collective cost left on the critical path is the final RS chunk.
"""

import math
import numpy as np

import concourse.bass as bass
import concourse.bacc as bacc
import concourse.tile as tile
import concourse.mybir as mybir
from concourse.masks import make_identity

P = 128
S = 1024
D = 4096
HD = 128
H = 32
C = 8
NB = D // P        # 32 blocks over the D contraction
SB = S // P        # 8 blocks over sequence
NCORES = 8
NEG = -1e9

NQC = 4            # ReduceScatter chunks over the sequence
QW = S // NQC      # rows per chunk
SBQ = QW // P      # sb blocks per chunk
SH = QW // NCORES  # rows each rank keeps per chunk

WA = 384           # A-slot attn width (3 heads)
WB = 128           # B-slot attn width
OA = WA            # ones column for A at 384
OB = WA + 1 + WB   # ones column for B at 513
VW = WA + WB + 2   # 514

F32 = mybir.dt.float32
BF16 = mybir.dt.bfloat16


def _decompose_31(nper):
    """Split clusters into 8 three-head chunks + 8 one-head chunks."""
    a = [n // 3 for n in nper]
    total = sum(a)
    if total < 8:
        return None
    cs_ = list(range(len(nper)))
    i = 0
    while total > 8:
        c = cs_[i % len(cs_)]
        if a[c] > 0:
            a[c] -= 1
            total -= 1
        i += 1
    b = [nper[c] - 3 * a[c] for c in range(len(nper))]
    if sum(b) != 8 or any(x < 0 for x in b):
        return None
    threes, ones = [], []
    for c in range(len(nper)):
        threes += [c] * a[c]
        ones += [c] * b[c]
    if len(threes) != 8 or len(ones) != 8:
        return None
    return threes, ones


def prep(inputs, compute_in_bf16_io=True):
    x = np.asarray(inputs["x"], np.float32)
    wq = np.asarray(inputs["wq"], np.float32)
    wk = np.asarray(inputs["wk"], np.float32)
    wv = np.asarray(inputs["wv"], np.float32)
    wo = np.asarray(inputs["wo"], np.float32)
    cos = np.asarray(inputs["freqs_cos"], np.float32)
    sin = np.asarray(inputs["freqs_sin"], np.float32)
    mask = np.asarray(inputs["mask"], np.float32)
    lab = np.asarray(inputs["cluster_assignment"]).astype(np.int64)
    rep = np.asarray(inputs["rep_heads"]).astype(np.int64)

    members = [[h for h in range(H) if lab[h] == c] for c in range(C)]
    nper = [len(m) for m in members]
    dec = _decompose_31(nper)
    assert dec is not None, "3+1 decomposition failed for this input"
    threes, ones = dec

    cursor = {c: 0 for c in range(C)}

    def take(c, k):
        i0 = cursor[c]
        cursor[c] += k
        return members[c][i0:i0 + k]

    slotsA = [(c, take(c, 3)) for c in threes]
    slotsB = [(c, take(c, 1)) for c in ones]
    assert all(cursor[c] == nper[c] for c in range(C))

    causal_mask = np.where(np.triu(np.ones((S, S), bool), k=1),
                           np.float32(NEG), np.float32(0.0))
    causal = np.array_equal(mask, causal_mask)

    import ml_dtypes
    bf = ml_dtypes.bfloat16

    def cvt(a):
        return np.ascontiguousarray(a).astype(bf)

    # device-layout packing: [P, NB*X] with row d = b*P + p, so every
    # big DMA is one contiguous per-partition segment
    def devpack(a):
        # a: [D, X] -> [P, (D//P)*X]
        X = a.shape[1]
        return np.ascontiguousarray(
            a.reshape(NB, P, X).transpose(1, 0, 2).reshape(P, NB * X))

    xT = cvt(devpack(x[0].T))
    cs = np.empty((HD, S), np.float32)
    sn = np.empty((HD, S), np.float32)
    cs[0::2] = cos.T
    cs[1::2] = cos.T
    sn[0::2] = -sin.T
    sn[1::2] = sin.T
    psw = np.zeros((HD, HD), np.float32)
    for i in range(HD):
        psw[i, i ^ 1] = 1.0
    kk = np.arange(P)[:, None]
    qq = np.arange(P)[None, :]
    tri = np.where(kk > qq, np.float32(NEG), np.float32(0.0))

    scale = 1.0 / math.sqrt(HD)

    in_maps = []
    for i in range(NCORES):
        cA, headsA = slotsA[i]
        cB, headsB = slotsB[i]
        wqkA = np.empty((D, 2 * HD), np.float32)
        wqkA[:, 0:HD] = wq[rep[cA] * HD:(rep[cA] + 1) * HD, :].T * scale
        wqkA[:, HD:2 * HD] = wk[rep[cA] * HD:(rep[cA] + 1) * HD, :].T
        wqkB = np.empty((D, 2 * HD), np.float32)
        wqkB[:, 0:HD] = wq[rep[cB] * HD:(rep[cB] + 1) * HD, :].T * scale
        wqkB[:, HD:2 * HD] = wk[rep[cB] * HD:(rep[cB] + 1) * HD, :].T
        wvT = np.empty((D, 4 * HD), np.float32)
        for j, h in enumerate(headsA + headsB):
            wvT[:, j * P:(j + 1) * P] = wv[h * HD:(h + 1) * HD, :].T
        # wo rows for my 4 heads, all 4096 output columns:
        # wavT[:, j, :] = wo[:, h*HD:(h+1)*HD].T  ([head-dim 128, D])
        wavT = np.empty((P, 4 * D), np.float32)
        for j, h in enumerate(headsA + headsB):
            wavT[:, j * D:(j + 1) * D] = wo[:, h * HD:(h + 1) * HD].T
        m = {
            "xT": xT, "wqkAT": cvt(devpack(wqkA)),
            "wqkBT": cvt(devpack(wqkB)), "wvT": cvt(devpack(wvT)),
            "woT": cvt(wavT),
            "cs": cs, "sn": sn, "psw": psw, "tri": tri,
        }
        if not causal:
            m["maskT"] = np.ascontiguousarray(mask.T)
        in_maps.append(m)

    meta = {"causal": causal, "bf16_io": True}
    return in_maps, meta


def assemble(results, meta):
    out = np.empty((1, S, D), np.float32)
    for c in range(NCORES):
        r = np.asarray(results[c]["out"], np.float32)
        for q in range(NQC):
            out[0, q * QW + c * SH:q * QW + (c + 1) * SH, :] = \
                r[q * SH:(q + 1) * SH, :]
    return out


def emit_body(nc, tc, tensors, meta, body_idx=0, chain=False):
    causal = meta["causal"]
    xT, wqkAT, wqkBT, wvT, woT = (
        tensors[k] for k in ("xT", "wqkAT", "wqkBT", "wvT", "woT"))
    cs, sn, psw, tri = (tensors[k] for k in ("cs", "sn", "psw", "tri"))
    out = tensors["out"]
    maskT = tensors.get("maskT")

    XG = 8
    GB = NB // XG

    with (
        tc.tile_pool(name="const", bufs=1) as constp,
        tc.tile_pool(name="persist", bufs=1) as persist,
        tc.tile_pool(name="ph1w", bufs=1) as ph1w,
        tc.tile_pool(name="dram", bufs=1, space="DRAM") as dram,
    ):
        ident = constp.tile([P, P], BF16)
        make_identity(nc, ident[:])
        trib = constp.tile([P, P], F32)
        nc.sync.dma_start(trib[:], tri[:])

        qk_dT = [persist.tile([P, S], BF16, name=f"qk{i}_{body_idx}")
                 for i in range(4)]    # qA, kA, qB, kB
        Vs = [persist.tile([P, VW], BF16, name=f"V{kb}_{body_idx}")
              for kb in range(SB)]
        # pT tiles shrunk to the causal width (col c maps to q = kb*P + c)
        pTw = [(S - kb * P if causal else S) for kb in range(SB)]
        pTa = [persist.tile([P, pTw[kb]], BF16, name=f"pTa{kb}_{body_idx}")
               for kb in range(SB)]
        pTb = [persist.tile([P, pTw[kb]], BF16, name=f"pTb{kb}_{body_idx}")
               for kb in range(SB)]
        # transposed attention outputs for all 8 row-blocks (4 head-blocks)
        aT_all = persist.tile([P, 4, S], BF16, name=f"aT_{body_idx}")

        junk_b = None
        if chain and body_idx > 0:
            junk = constp.tile([P, 16], BF16, name="junk")
            nc.sync.dma_start(junk[:], out[0:P, 0:16])
            junk_b = constp.tile([P, 16], BF16, name="junkb")
            nc.vector.tensor_scalar_mul(junk_b[:], junk[:], 0.0)

        def chain_dep(ap2d):
            if junk_b is not None:
                nc.vector.tensor_copy(ap2d, junk_b[:])

        # ================= projections (scoped pools) ===================
        with (
            tc.tile_pool(name="wqkp", bufs=1) as wqkp,
            tc.tile_pool(name="ropep", bufs=2) as ropep,
            tc.tile_pool(name="psproj", bufs=1, space="PSUM") as psprojp,
            tc.tile_pool(name="psrope", bufs=2, space="PSUM") as psropep,
        ):
            wqkAb = wqkp.tile([P, NB, 2 * HD], BF16)
            wqkBb = wqkp.tile([P, NB, 2 * HD], BF16)
            chain_dep(wqkAb[:, 0, :16])
            chain_dep(wqkBb[:, 0, :16])
            wvb = ph1w.tile([P, NB, 4 * HD], BF16)
            chain_dep(wvb[:, 0, :16])
            xbs = []
            for g in range(XG):
                # interleave A-pair weight chunks with x groups so the
                # first projection matmul can start after ~1.3 MB of DMA
                nc.sync.dma_start(
                    wqkAb[:, g * GB:(g + 1) * GB, :],
                    wqkAT[:, g * GB * 2 * HD:(g + 1) * GB * 2 * HD])
                xg = ph1w.tile([P, GB, S], BF16, name=f"xb{g}")
                chain_dep(xg[:, 0, :16])
                nc.sync.dma_start(
                    xg[:], xT[:, g * GB * S:(g + 1) * GB * S])
                xbs.append(xg)
                if g == 0:
                    csb = ropep.tile([P, S], F32, tag="csb")
                    nc.sync.dma_start(csb[:], cs[:])
                    snb = ropep.tile([P, S], F32, tag="snb")
                    nc.sync.dma_start(snb[:], sn[:])
                    pswf = ropep.tile([P, P], F32, tag="pswf")
                    nc.sync.dma_start(pswf[:], psw[:])
                    pswb = ropep.tile([P, P], BF16, tag="pswb")
                    nc.vector.tensor_copy(pswb[:], pswf[:])
                if g == 3:
                    nc.sync.dma_start(wqkBb[:], wqkBT[:, :])
                nc.sync.dma_start(
                    wvb[:, g * GB:(g + 1) * GB, :],
                    wvT[:, g * GB * 4 * HD:(g + 1) * GB * 4 * HD])

            def xb(db):
                return xbs[db // GB][:, db % GB, :]

            def proj_pair(base):
                # base=0 -> qA,kA ; base=2 -> qB,kB. db-outer so the
                # matmuls start as soon as the first x group lands.
                wb = wqkAb if base == 0 else wqkBb
                pss = [psprojp.tile([P, 512], F32, tag=f"pp{j}",
                                    name=f"pp{base}_{j}_{body_idx}")
                       for j in range(4)]  # q ch0, q ch1, k ch0, k ch1
                for db in range(NB):
                    for pj in range(2):       # q, k
                        for ch in range(2):
                            nc.tensor.matmul(
                                pss[pj * 2 + ch][:],
                                wb[:, db, pj * HD:(pj + 1) * HD],
                                xb(db)[:, ch * 512:(ch + 1) * 512],
                                start=(db == 0), stop=(db == NB - 1))
                return pss

            def rope(pss, base):
                for pj in range(2):
                    dst = qk_dT[base + pj]
                    raw = ropep.tile([P, S], BF16, tag="qraw")
                    for ch in range(2):
                        nc.vector.tensor_copy(raw[:, ch * 512:(ch + 1) * 512],
                                              pss[pj * 2 + ch][:])
                    for ch in range(2):
                        csl = slice(ch * 512, (ch + 1) * 512)
                        ps2 = psropep.tile([P, 512], F32, tag="pswp")
                        nc.tensor.matmul(ps2[:], pswb[:], raw[:, csl],
                                         start=True, stop=True)
                        t1 = ropep.tile([P, 512], F32, tag="rope1")
                        nc.vector.tensor_tensor(t1[:], ps2[:], snb[:, csl],
                                                op=mybir.AluOpType.mult)
                        t2 = ropep.tile([P, 512], F32, tag="rope2")
                        nc.vector.tensor_tensor(t2[:], raw[:, csl],
                                                csb[:, csl],
                                                op=mybir.AluOpType.mult)
                        nc.vector.tensor_tensor(dst[:, csl], t1[:], t2[:],
                                                op=mybir.AluOpType.add)

            pssA = proj_pair(0)
            rope(pssA, 0)
            pssB = proj_pair(2)
            rope(pssB, 2)

        # wo weights for my 4 heads x all D columns; the 4MB DMA streams
        # in the background while the attention phase runs on PE
        wow = tc.tile_pool(name="wow", bufs=1)
        wowp = wow.__enter__()
        wob = wowp.tile([P, 4, D], BF16)
        chain_dep(wob[:, 0, :16])
        nc.sync.dma_start(wob[:], woT[:])

        # ============== scores_A + interleaved V/scores_B/AV ============
        with (
            tc.tile_pool(name="ph3", bufs=2) as ph3,
            tc.tile_pool(name="psv", bufs=2, space="PSUM") as psvp,
            tc.tile_pool(name="pssc", bufs=2, space="PSUM") as psscp,
            tc.tile_pool(name="pav", bufs=1, space="PSUM") as pavp,
            tc.tile_pool(name="ptr", bufs=2, space="PSUM") as ptrp,
        ):
            def scores(qi, ki_, pTs, kb):
                q0 = kb * P if causal else 0
                if maskT is not None:
                    mrow = ph3.tile([P, S], F32, tag="mrow")
                    nc.sync.dma_start(mrow[:], maskT[kb * P:(kb + 1) * P, :])
                c0 = q0
                while c0 < S:
                    c1 = min(c0 + 512, S)
                    ps = psscp.tile([P, 512], F32, tag="psc")
                    nc.tensor.matmul(ps[:, :c1 - c0],
                                     qk_dT[ki_][:, kb * P:(kb + 1) * P],
                                     qk_dT[qi][:, c0:c1],
                                     start=True, stop=True)
                    if maskT is not None:
                        nc.vector.tensor_tensor(
                            ps[:, :c1 - c0], ps[:, :c1 - c0],
                            mrow[:, c0:c1], op=mybir.AluOpType.add)
                    elif causal and c0 == q0:
                        nc.vector.tensor_tensor(
                            ps[:, :P], ps[:, :P], trib[:],
                            op=mybir.AluOpType.add)
                    nc.scalar.activation(
                        pTs[kb][:, c0 - (kb * P if causal else 0):
                                c1 - (kb * P if causal else 0)],
                        ps[:, :c1 - c0],
                        mybir.ActivationFunctionType.Exp)
                    c0 = c1

            for kb in range(SB):
                scores(0, 1, pTa, kb)
            scores(2, 3, pTb, 0)

            for sb in range(SB):
                # V projection for this sequence block
                psv = psvp.tile([P, 512], F32, tag="pv")
                for db in range(NB):
                    nc.tensor.matmul(
                        psv[:], xb(db)[:, sb * P:(sb + 1) * P],
                        wvb[:, db, :],
                        start=(db == 0), stop=(db == NB - 1))
                nc.vector.tensor_copy(Vs[sb][:, 0:WA], psv[:, 0:WA])
                nc.vector.tensor_copy(Vs[sb][:, OA + 1:OA + 1 + WB],
                                      psv[:, WA:WA + WB])
                nc.vector.memset(Vs[sb][:, OA:OA + 1], 1.0)
                nc.vector.memset(Vs[sb][:, OB:OB + 1], 1.0)

                # B scores one block ahead (hide the exp latency)
                if sb + 1 < SB:
                    scores(2, 3, pTb, sb + 1)

                # AV for q block sb (causal: ki <= sb)
                kmax = sb + 1 if causal else SB
                psA = pavp.tile([P, WA + 1], F32, tag="pava",
                                name=f"pava{sb}_{body_idx}")
                psB = pavp.tile([P, WB + 1], F32, tag="pavb",
                                name=f"pavb{sb}_{body_idx}")
                for ki in range(kmax):
                    o = ki * P if causal else 0
                    ltA = pTa[ki][:, sb * P - o:(sb + 1) * P - o]
                    ltB = pTb[ki][:, sb * P - o:(sb + 1) * P - o]
                    nc.tensor.matmul(psA[:], ltA, Vs[ki][:, 0:WA + 1],
                                     start=(ki == 0), stop=(ki == kmax - 1))
                    nc.tensor.matmul(psB[:], ltB, Vs[ki][:, OA + 1:OB + 1],
                                     start=(ki == 0), stop=(ki == kmax - 1))
                rLA = ph3.tile([P, 1], F32, tag="rLA")
                nc.vector.reciprocal(rLA[:], psA[:, WA:WA + 1])
                rLB = ph3.tile([P, 1], F32, tag="rLB")
                nc.vector.reciprocal(rLB[:], psB[:, WB:WB + 1])
                attn = ph3.tile([P, 512], BF16, tag="attn")
                nc.vector.tensor_scalar_mul(attn[:, 0:WA], psA[:, :WA],
                                            rLA[:])
                nc.vector.tensor_scalar_mul(attn[:, WA:512], psB[:, :WB],
                                            rLB[:])
                for blk in range(4):
                    pst = ptrp.tile([P, P], BF16, tag="ptr")
                    nc.tensor.transpose(pst[:], attn[:, blk * P:(blk + 1) * P],
                                        ident[:])
                    nc.vector.tensor_copy(
                        aT_all[:, blk, sb * P:(sb + 1) * P], pst[:])

        # ========== wo partials (own 4 heads, all D cols) + RS ==========
        with (
            tc.tile_pool(name="ph5", bufs=2) as ph5,
            tc.tile_pool(name="po", bufs=1, space="PSUM") as pop,
        ):
            NBANK = 8
            for q in range(NQC):
                chunk = dram.tile([QW, D], BF16, name=f"pc{body_idx}_{q}")
                for s_ in range(SBQ):
                    sb = q * SBQ + s_
                    pos = [pop.tile([P, 512], F32, tag=f"po{b}",
                                    name=f"po{b}_{sb}_{body_idx}")
                           for b in range(NBANK)]
                    for j in range(4):
                        for b in range(NBANK):
                            nc.tensor.matmul(
                                pos[b][:],
                                aT_all[:, j, sb * P:(sb + 1) * P],
                                wob[:, j, b * 512:(b + 1) * 512],
                                start=(j == 0), stop=(j == 3))
                    ot = ph5.tile([P, D], BF16, tag="ot")
                    for b in range(NBANK):
                        nc.vector.tensor_copy(
                            ot[:, b * 512:(b + 1) * 512], pos[b][:])
                    nc.sync.dma_start(chunk[s_ * P:(s_ + 1) * P, :], ot[:])
                rso = dram.tile([SH, D], BF16, name=f"rso{body_idx}_{q}")
                if meta.get("nocoll"):
                    nc.sync.dma_start(rso[:], chunk[0:SH, :])
                else:
                    nc.gpsimd.collective_compute(
                        "ReduceScatter", mybir.AluOpType.add,
                        replica_groups=[list(range(NCORES))],
                        ins=[chunk[:].opt()], outs=[rso[:].opt()])
                nc.sync.dma_start(out[q * SH:(q + 1) * SH, :], rso[:])
        wow.__exit__(None, None, None)


def build_kernel(meta, repeat=1, chain=True):
    nc = bacc.Bacc("TRN2", target_bir_lowering=False, debug=False,
                   num_devices=NCORES)
    tensors = {
        "xT": nc.dram_tensor("xT", [P, NB * S], BF16, kind="ExternalInput"),
        "wqkAT": nc.dram_tensor("wqkAT", [P, NB * 2 * HD], BF16,
                                kind="ExternalInput"),
        "wqkBT": nc.dram_tensor("wqkBT", [P, NB * 2 * HD], BF16,
                                kind="ExternalInput"),
        "wvT": nc.dram_tensor("wvT", [P, NB * 4 * HD], BF16,
                              kind="ExternalInput"),
        "woT": nc.dram_tensor("woT", [P, 4 * D], BF16,
                              kind="ExternalInput"),
        "cs": nc.dram_tensor("cs", [HD, S], F32, kind="ExternalInput"),
        "sn": nc.dram_tensor("sn", [HD, S], F32, kind="ExternalInput"),
        "psw": nc.dram_tensor("psw", [HD, HD], F32, kind="ExternalInput"),
        "tri": nc.dram_tensor("tri", [P, P], F32, kind="ExternalInput"),
        "out": nc.dram_tensor("out", [P, D], BF16, kind="ExternalOutput"),
    }
    if not meta["causal"]:
        tensors["maskT"] = nc.dram_tensor("maskT", [S, S], F32,
                                          kind="ExternalInput")
    with tile.TileContext(nc) as tc:
        if repeat == 0:
            with tc.tile_pool(name="z", bufs=1) as zp:
                zt = zp.tile([P, D], BF16)
                nc.vector.memset(zt[:], 0.0)
                nc.sync.dma_start(tensors["out"][:], zt[:])
        else:
            for r in range(repeat):
                emit_body(nc, tc, tensors, meta, body_idx=r, chain=chain)
    nc.compile()
    return nc


def kernel(**inputs):
    import numpy as _np
    np_inputs = {k: (_np.asarray(v) if not _np.isscalar(v) else v)
                 for k, v in inputs.items()}
    in_maps, meta = prep(np_inputs)
    nc = build_kernel(meta, repeat=1, chain=False)
    from concourse import bass_utils
    res = bass_utils.run_bass_kernel_spmd(
        nc, in_maps, core_ids=list(range(NCORES)))
    return assemble(res.results, meta)
